# revision 59
# baseline (speedup 1.0000x reference)
"""AttLoc kernel for 8 TRN2 NeuronCores.

Computation (per example b):
  pre   = enc @ W_enc + b_enc                  [T, A]
  conv  = conv1d(att_prev, conv_w, pad=K)      [C, T]
  attc  = W_att^T @ conv                       [A, T] (transposed view)
  dec_b = dec_z @ W_dec                        [A]
  s     = tanh(pre^T + attc + dec_b + b_enc)   [A, T]
  e     = g^T @ s                              [T]
  w     = softmax(2*e)                         [T]
  c     = (w @ enc) @ W_o + b_o                [O]
returns (c [B,O], w [B,T])

Sharding: data-parallel over batch: 8 cores x 4 examples. Weights replicated.

Per-core dataflow:
  enc (f32, HBM) --SWDGE cast-DMA--> nat_bf16 [128t, ex, tc, 512e]
  nat --xbar DMA transpose--> encT [128e, ex, ec, t]
  PE: pretanh[a_tile, t_bank] = sum_ec W_enc^T @ encT  (+ conv term, row-packed)
  ACT: tanh with per-partition bias (b_enc + dec_b), out bf16
  PE: e = g . s  (col-packed over examples), softmax on DVE/ACT,
  PE: c = w @ enc (col-packed), out = c @ W_o (+ b_o via extra contraction row)
"""

import sys

sys.path.insert(0, "/opt/trn_rl_repo")

import numpy as np

import concourse.bass as bass
import concourse.mybir as mybir
import concourse.tile as tile
from concourse import bacc
from concourse.bass import ds, ts

# problem dims (hardcoded; kernel.py must be self-contained)
B, T = 32, 1500
E, D, A, C, K, O = 512, 512, 512, 10, 100, 512
SCALING = 2.0
NCORES = 8
BE = B // NCORES  # 4 examples per core

F32 = mybir.dt.float32
BF16 = mybir.dt.bfloat16

P = 128
TC = 12          # t chunks of 128 (11 full + partial 92); padded alloc
T_PAD = TC * P   # 1536
T_LAST = T - 11 * P  # 92
EC = E // P      # 4
AT = A // P      # 4 a tiles
T_BANKS = [(0, 512), (512, 512), (1024, 476)]  # psum-bank sized t slices
KK = 2 * K + 1   # 201 conv taps
K1N = 2          # tap chunks of 128 (128 + 73)
X_FREE = T + P   # 1628: X2b[k2, t'] = ap_pad[t' + k2], t' in [0, X_FREE)
AP_LEN = X_FREE + P  # padded att_prev row length 1756 (even)


def _build(nc: bacc.Bacc):
    # ---- DRAM I/O (per-core shard shapes) ----
    enc = nc.dram_tensor("enc_pad", [BE, T, E], F32, kind="ExternalInput").ap()
    enc_len = nc.dram_tensor("enc_len", [BE], mybir.dt.int32, kind="ExternalInput").ap()  # noqa: F841 (unused by math)
    dec_z = nc.dram_tensor("dec_z", [BE, D], F32, kind="ExternalInput").ap()
    att_prev = nc.dram_tensor("att_prev", [BE, T], F32, kind="ExternalInput").ap()
    W_enc = nc.dram_tensor("W_enc", [E, A], F32, kind="ExternalInput").ap()
    b_enc = nc.dram_tensor("b_enc", [A], F32, kind="ExternalInput").ap()
    W_dec = nc.dram_tensor("W_dec", [D, A], F32, kind="ExternalInput").ap()
    W_att = nc.dram_tensor("W_att", [C, A], F32, kind="ExternalInput").ap()
    conv_w = nc.dram_tensor("conv_w", [C, 1, KK], F32, kind="ExternalInput").ap()
    gvec_w = nc.dram_tensor("gvec_w", [A], F32, kind="ExternalInput").ap()
    W_o = nc.dram_tensor("W_o", [E, O], F32, kind="ExternalInput").ap()
    b_o = nc.dram_tensor("b_o", [O], F32, kind="ExternalInput").ap()
    out_c = nc.dram_tensor("out_c", [BE, O], F32, kind="ExternalOutput").ap()
    out_w = nc.dram_tensor("out_w", [BE, T], F32, kind="ExternalOutput").ap()
    # small DRAM scratch for partition<->free relayouts
    wtmp = nc.dram_tensor("wtmp", [BE, T_PAD], BF16, kind="Internal").ap()
    aptmp = nc.dram_tensor("aptmp", [1, BE, AP_LEN], BF16, kind="Internal").ap()
    ctmp = nc.dram_tensor("ctmp", [BE, E], F32, kind="Internal").ap()
    dtmp = nc.dram_tensor("dtmp", [BE, A], F32, kind="Internal").ap()

    with tile.TileContext(nc) as tc, \
         tc.tile_pool(name="persist", bufs=1) as pp, \
         tc.tile_pool(name="work", bufs=3) as wp, \
         tc.tile_pool(name="stgp", bufs=2) as stgp, \
         tc.tile_pool(name="ppre", bufs=4, space="PSUM") as ppre, \
         tc.tile_pool(name="pbig", bufs=1, space="PSUM") as pbig, \
         tc.tile_pool(name="pmisc", bufs=1, space="PSUM") as pmisc:

        # ================= prologue: weights =================
        # W_enc as lhsT [e, a] -> [128, ec, A] bf16
        W_enc_b = pp.tile([P, EC, A], BF16, tag="W_enc_b")
        nc.gpsimd.dma_start(W_enc_b[:], W_enc.rearrange("(ec p) a -> p ec a", p=P))
        # W_dec lhsT [d, a]
        W_dec_b = pp.tile([P, EC, A], BF16, tag="W_dec_b")
        nc.gpsimd.dma_start(W_dec_b[:], W_dec.rearrange("(dc p) a -> p dc a", p=P))
        # W_o rhs [e, o]
        W_o_b = pp.tile([P, EC, O], BF16, tag="W_o_b")
        nc.gpsimd.dma_start(W_o_b[:], W_o.rearrange("(ec p) o -> p ec o", p=P))
        # W_att padded to 32 contraction rows (rows C..32 zero)
        W_att_b = pp.tile([32, A], BF16, tag="W_att_b")
        nc.any.memzero(W_att_b[:])
        nc.gpsimd.dma_start(W_att_b[:C, :], W_att)
        # conv taps [k2, k1, c] zero-padded beyond 201; k = 128*k1 + k2
        taps = pp.tile([P, K1N, C], BF16, tag="taps")
        nc.any.memzero(taps[:])
        with nc.allow_non_contiguous_dma(reason="small conv tap relayout"):
            cw = conv_w.rearrange("c one k -> c (one k)")
            nc.gpsimd.dma_start(
                taps[:, 0, :], cw[:, 0:P].rearrange("c k2 -> k2 c")
            )
            nc.gpsimd.dma_start(
                taps[: KK - P, 1, :], cw[:, P:KK].rearrange("c k2 -> k2 c")
            )
        # g vector as [128, at] bf16 (lhsT columns)
        g_b = pp.tile([P, AT], BF16, tag="g_b")
        with nc.allow_non_contiguous_dma(reason="small g relayout"):
            nc.gpsimd.dma_start(g_b[:], gvec_w.rearrange("(at p) -> p at", p=P))
        # b_enc per-partition [128, at] f32
        b_enc_sb = pp.tile([P, AT], F32, tag="b_enc_sb")
        with nc.allow_non_contiguous_dma(reason="small b_enc relayout"):
            nc.scalar.dma_start(b_enc_sb[:], b_enc.rearrange("(at p) -> p at", p=P))
        # b_o replicated to BE rows for the final bias add (stride-0 read)
        b_o4 = pp.tile([BE, O], F32, tag="b_o4")
        with nc.allow_non_contiguous_dma(reason="broadcast read"):
            nc.scalar.dma_start(
                b_o4[:],
                bass.AP(tensor=b_o.tensor, offset=0, ap=[[0, BE], [1, O]]),
            )
        # dec_z^T [d, dc, ex] bf16, padded to 128 lhsT columns (cols BE..128
        # zero) — small-M matmuls mis-lower on HW, so keep M=128
        dec_zT = pp.tile([P, EC, P], BF16, tag="dec_zT")
        nc.any.memzero(dec_zT[:])
        with nc.allow_non_contiguous_dma(reason="small dec relayout"):
            for ex in range(BE):
                nc.gpsimd.dma_start(
                    dec_zT[:, :, ex], dec_z[ex].rearrange("(dc p) -> p dc", p=P)
                )

        # ================= prologue: att_prev -> X2b =================
        # build the padded row in SBUF, bounce to DRAM, then ONE windowed DMA
        # reads the 128 overlapping shifted copies back
        ap_row = pp.tile([1, BE, AP_LEN], BF16, tag="ap_row")
        nc.any.memzero(ap_row[:])
        nc.gpsimd.dma_start(ap_row[0, :, K : K + T], att_prev)
        nc.scalar.dma_start(aptmp, ap_row[:])
        X2b = pp.tile([P, BE, X_FREE], BF16, tag="X2b")
        windows = bass.AP(
            tensor=aptmp.tensor,
            offset=0,
            ap=[[1, P], [AP_LEN, BE], [1, X_FREE]],
        )
        with nc.allow_non_contiguous_dma(reason="overlapping shifted windows"):
            nc.scalar.dma_start(X2b[:], windows)

        # dec_b = dec_z @ W_dec as [ex, a] (M=4 padded to 128), then
        # transpose via DRAM roundtrip into per-partition bias columns
        W_dec_b2 = pp.tile([P, EC, A], BF16, tag="W_dec_b2")
        nc.vector.tensor_copy(W_dec_b2[:], W_dec_b[:])
        dec_ps = pmisc.tile([P, 512], F32, tag="misc")
        for dc in range(EC):
            nc.tensor.matmul(
                dec_ps[:, :A],
                dec_zT[:, dc, :],
                W_dec_b2[:, dc, :],
                start=(dc == 0),
                stop=(dc == EC - 1),
            )
        decb_sb = pp.tile([BE, A], F32, tag="decb_sb")
        nc.vector.tensor_copy(decb_sb[:], dec_ps[:BE, :A])
        nc.scalar.dma_start(dtmp, decb_sb[:])
        dec_col = pp.tile([P, BE, AT], F32, tag="dec_col")
        with nc.allow_non_contiguous_dma(reason="small dec_b relayout"):
            nc.scalar.dma_start(
                dec_col[:], dtmp.rearrange("b (at p) -> p b at", p=P)
            )
        # bias[p, ex, at] = b_enc + dec_b
        bias_sb = pp.tile([P, BE, AT], F32, tag="bias_sb")
        nc.vector.tensor_tensor(
            bias_sb[:],
            dec_col[:],
            b_enc_sb[:, None, :].to_broadcast((P, BE, AT)),
            mybir.AluOpType.add,
        )

        # ================= per-example pipeline =================
        # ex-outer: each example's loads/transpose/compute/softmax/context
        # overlap the next example's loads. Within (ex, at), the 3 t-banks
        # share each stationary W_enc chunk.
        conv_sb = pp.tile([32, BE, T_PAD], BF16, tag="conv_sb")
        nc.any.memzero(conv_sb[:])
        e_ps = pbig.tile([P, T_PAD], F32, tag="big")
        su3 = pp.tile([P, 4], F32, tag="su3")
        w_b = pp.tile([P, T_PAD], BF16, tag="w_b")
        nc.any.memzero(w_b[:, T:])
        c_sb = pp.tile([P, E], F32, tag="c_sb")
        nats, encTs = [], []
        for ex in range(BE):
            r = slice(32 * ex, 32 * ex + 1)
            # ---- load (HWDGE, f32) + cast (DVE) + transpose ----
            nat = pp.tile([P, EC, T_PAD], BF16, tag=f"nat{ex}")
            nc.any.memzero(nat[:, :, 11 * P :])
            for (tc0, ntc) in [(0, 4), (4, 4), (8, 3)]:
                stg = stgp.tile([P, 4, E], F32, tag="stg")
                nc.sync.dma_start(
                    stg[:, :ntc, :],
                    enc[ex, tc0 * P : (tc0 + ntc) * P, :].rearrange(
                        "(tc p) e -> p tc e", p=P
                    ),
                )
                nc.vector.tensor_copy(
                    nat[:, :, tc0 * P : (tc0 + ntc) * P].rearrange(
                        "p ec (tc i) -> p ec tc i", i=P
                    ),
                    stg[:, :ntc, :].rearrange("p tc (ec i) -> p ec tc i", i=P),
                )
            stg2 = stgp.tile([P, E], F32, tag="stg2")
            nc.sync.dma_start(stg2[:T_LAST, :], enc[ex, 11 * P : T, :])
            nc.vector.tensor_copy(
                nat[:T_LAST, :, 11 * P :].rearrange("p ec i -> p ec i"),
                stg2[:T_LAST, :].rearrange("p (ec i) -> p ec i", i=P),
            )
            encT = pp.tile([P, EC, T_PAD], BF16, tag=f"encT{ex}")
            nc.sync.dma_start_transpose(
                encT.rearrange("p ec (tc i) -> p (ec tc) i", i=P),
                nat.rearrange("p ec f -> p (ec f)"),
            )
            nats.append(nat)
            encTs.append(encT)
            # ---- conv stage 1 ----
            conv_ps = ppre.tile([P, 512], F32, tag="pre", name=f"cps{ex}")
            for (t0, bw) in T_BANKS:
                for k1 in range(K1N):
                    nc.tensor.matmul(
                        conv_ps[:C, :bw],
                        taps[:, k1, :],
                        X2b[:, ex, P * k1 + t0 : P * k1 + t0 + bw],
                        start=(k1 == 0),
                        stop=(k1 == K1N - 1),
                    )
                nc.vector.tensor_copy(
                    conv_sb[:C, ex, t0 : t0 + bw], conv_ps[:C, :bw]
                )
            # ---- main: pre-tanh, tanh, e ----
            for at in range(AT):
                pres = [
                    ppre.tile([P, 512], F32, tag="pre", name=f"pre{ex}_{at}_{x}")
                    for x in range(len(T_BANKS))
                ]
                for ec in range(EC):
                    for bi, (t0, bw) in enumerate(T_BANKS):
                        nc.tensor.matmul(
                            pres[bi][:, :bw],
                            W_enc_b[:, ec, at * P : (at + 1) * P],
                            encT[:, ec, t0 : t0 + bw],
                            start=(ec == 0),
                            stop=False,
                        )
                for bi, (t0, bw) in enumerate(T_BANKS):
                    nc.tensor.matmul(
                        pres[bi][:, :bw],
                        W_att_b[:, at * P : (at + 1) * P],
                        conv_sb[:, ex, t0 : t0 + bw],
                        start=False,
                        stop=True,
                    )
                for bi, (t0, bw) in enumerate(T_BANKS):
                    s_t = wp.tile([P, 512], BF16, tag="s_t")
                    nc.scalar.activation(
                        s_t[:, :bw],
                        pres[bi][:, :bw],
                        mybir.ActivationFunctionType.Tanh,
                        bias=bias_sb[:, ex, at : at + 1],
                    )
                    nc.tensor.matmul(
                        e_ps[r, t0 : t0 + bw],
                        g_b[:, at : at + 1],
                        s_t[:, :bw],
                        start=(at == 0),
                        stop=(at == AT - 1),
                        tile_position=(0, 32 * ex),
                    )
            # ---- softmax numerator (unnormalized; |e|<=18 so f32-safe) ----
            for bi, (t0, bw) in enumerate(T_BANKS):
                nc.scalar.activation(
                    w_b[r, t0 : t0 + bw],
                    e_ps[r, t0 : t0 + bw],
                    mybir.ActivationFunctionType.Exp,
                    scale=SCALING,
                    accum_out=su3[r, bi : bi + 1],
                )
            # ---- context on unnormalized w ----
            nc.scalar.dma_start(wtmp[ex, :][None, :], w_b[r, :])
            w_colT = pp.tile([P, TC], BF16, tag=f"w_colT{ex}")
            with nc.allow_non_contiguous_dma(reason="small w relayout"):
                nc.scalar.dma_start(
                    w_colT[:], wtmp[ex].rearrange("(tc p) -> p tc", p=P)
                )
            c_ps = pmisc.tile([P, 512], F32, tag="misc", name=f"c_ps{ex}")
            for tcb in range(TC):
                nc.tensor.matmul(
                    c_ps[r, :E],
                    w_colT[:, tcb : tcb + 1],
                    nat[:, :, tcb * P : (tcb + 1) * P],
                    start=(tcb == 0),
                    stop=(tcb == TC - 1),
                    tile_position=(0, 32 * ex),
                )
            nc.vector.tensor_tensor(
                su3[r, 3:4], su3[r, 0:1], su3[r, 1:2], mybir.AluOpType.add
            )
            nc.vector.tensor_tensor(
                su3[r, 3:4], su3[r, 3:4], su3[r, 2:3], mybir.AluOpType.add
            )
            # 1/su via exp(-ln(su)) on ACT (DVE has no divide/cheap reciprocal)
            lns = wp.tile([P, 1], F32, tag="lns")
            nc.scalar.activation(
                lns[r, :], su3[r, 3:4], mybir.ActivationFunctionType.Ln
            )
            rsu = pp.tile([P, 1], F32, tag=f"rsu{ex}")
            nc.scalar.activation(
                rsu[r, :], lns[r, :], mybir.ActivationFunctionType.Exp,
                scale=-1.0,
            )
            nc.vector.tensor_tensor(
                c_sb[r, :],
                c_ps[r, :E],
                rsu[r, 0:1].to_broadcast((1, E)),
                mybir.AluOpType.mult,
            )
            nc.scalar.dma_start(ctmp[ex, :][None, :], c_sb[r, :])
            # normalized w output: numerator (bf16) * 1/su
            w_all = pp.tile([P, T], F32, tag="w_all")
            nc.vector.tensor_tensor(
                w_all[r, :],
                w_b[r, :T],
                rsu[r, 0:1].to_broadcast((1, T)),
                mybir.AluOpType.mult,
            )
            nc.scalar.dma_start(out_w[ex, :][None, :], w_all[r, :])

        cT = pp.tile([P, BE, EC], F32, tag="cT")
        with nc.allow_non_contiguous_dma(reason="small c relayout"):
            nc.scalar.dma_start(
                cT[:], ctmp.rearrange("b (ec p) -> p b ec", p=P)
            )
        cT_b = pp.tile([P, EC, P], BF16, tag="cT_b")
        nc.any.memzero(cT_b[:])
        nc.vector.tensor_copy(
            cT_b[:, :, :BE], cT.rearrange("p b e -> p e b")
        )

        # ================= out = c @ W_o + b_o =================
        W_o_b2 = pp.tile([P, EC, O], BF16, tag="W_o_b2")
        nc.vector.tensor_copy(W_o_b2[:], W_o_b[:])
        o_ps = pmisc.tile([P, 512], F32, tag="misc")
        for ec in range(EC):
            nc.tensor.matmul(
                o_ps[:, :O],
                cT_b[:, ec, :],
                W_o_b2[:, ec, :],
                start=(ec == 0),
                stop=(ec == EC - 1),
            )
        o_sb = pp.tile([BE, O], F32, tag="o_sb")
        nc.vector.tensor_tensor(
            o_sb[:], o_ps[:BE, :O], b_o4[:], mybir.AluOpType.add
        )
        nc.scalar.dma_start(out_c[:], o_sb[:])

    nc.compile()
    return nc


_CACHE = {}


def _get_nc():
    if "nc" not in _CACHE:
        nc = bacc.Bacc(
            "TRN2",
            target_bir_lowering=False,
            debug=False,
            enable_asserts=False,
            num_devices=NCORES,
        )
        _build(nc)
        _CACHE["nc"] = nc
    return _CACHE["nc"]


def make_in_maps(inputs):
    in_maps = []
    for i in range(NCORES):
        sl = slice(i * BE, (i + 1) * BE)
        m = {
            "enc_pad": np.ascontiguousarray(inputs["enc_pad"][sl], dtype=np.float32),
            "enc_len": np.ascontiguousarray(inputs["enc_len"][sl], dtype=np.int32),
            "dec_z": np.ascontiguousarray(inputs["dec_z"][sl], dtype=np.float32),
            "att_prev": np.ascontiguousarray(inputs["att_prev"][sl], dtype=np.float32),
            "W_enc": np.ascontiguousarray(inputs["W_enc"], dtype=np.float32),
            "b_enc": np.ascontiguousarray(inputs["b_enc"], dtype=np.float32),
            "W_dec": np.ascontiguousarray(inputs["W_dec"], dtype=np.float32),
            "W_att": np.ascontiguousarray(inputs["W_att"], dtype=np.float32),
            "conv_w": np.ascontiguousarray(inputs["conv_w"], dtype=np.float32),
            "gvec_w": np.ascontiguousarray(inputs["gvec_w"], dtype=np.float32),
            "W_o": np.ascontiguousarray(inputs["W_o"], dtype=np.float32),
            "b_o": np.ascontiguousarray(inputs["b_o"], dtype=np.float32),
        }
        in_maps.append(m)
    return in_maps


def kernel(**inputs):
    from concourse.bass_utils import run_bass_kernel_spmd

    nc = _get_nc()
    in_maps = make_in_maps({k: np.asarray(v) for k, v in inputs.items()})
    res = run_bass_kernel_spmd(nc, in_maps, core_ids=list(range(NCORES)))
    c = np.concatenate([r["out_c"] for r in res.results], axis=0)
    w = np.concatenate([r["out_w"] for r in res.results], axis=0)
    return (c, w)


if __name__ == "__main__":
    rng = np.random.default_rng(0)
    fake = {
        "enc_pad": rng.standard_normal((B, T, E), dtype=np.float32),
        "enc_len": np.full((B,), T, dtype=np.int32),
        "dec_z": rng.standard_normal((B, D), dtype=np.float32),
        "att_prev": rng.random((B, T), dtype=np.float32) / T,
        "W_enc": rng.standard_normal((E, A), dtype=np.float32) / np.sqrt(E),
        "b_enc": np.zeros((A,), dtype=np.float32),
        "W_dec": rng.standard_normal((D, A), dtype=np.float32) / np.sqrt(D),
        "W_att": rng.standard_normal((C, A), dtype=np.float32) / np.sqrt(C),
        "conv_w": rng.standard_normal((C, 1, KK), dtype=np.float32) * 0.05,
        "gvec_w": rng.standard_normal((A,), dtype=np.float32) / np.sqrt(A),
        "W_o": rng.standard_normal((E, O), dtype=np.float32) / np.sqrt(E),
        "b_o": np.zeros((O,), dtype=np.float32),
    }
    c, w = kernel(**fake)
    print("c", c.shape, c.dtype, "w", w.shape, w.dtype)


# revision 60
# speedup vs baseline: 1.0174x; 1.0174x over previous
"""AttLoc kernel for 8 TRN2 NeuronCores.

Computation (per example b):
  pre   = enc @ W_enc + b_enc                  [T, A]
  conv  = conv1d(att_prev, conv_w, pad=K)      [C, T]
  attc  = W_att^T @ conv                       [A, T] (transposed view)
  dec_b = dec_z @ W_dec                        [A]
  s     = tanh(pre^T + attc + dec_b + b_enc)   [A, T]
  e     = g^T @ s                              [T]
  w     = softmax(2*e)                         [T]
  c     = (w @ enc) @ W_o + b_o                [O]
returns (c [B,O], w [B,T])

Sharding: data-parallel over batch: 8 cores x 4 examples. Weights replicated.

Per-core dataflow:
  enc (f32, HBM) --SWDGE cast-DMA--> nat_bf16 [128t, ex, tc, 512e]
  nat --xbar DMA transpose--> encT [128e, ex, ec, t]
  PE: pretanh[a_tile, t_bank] = sum_ec W_enc^T @ encT  (+ conv term, row-packed)
  ACT: tanh with per-partition bias (b_enc + dec_b), out bf16
  PE: e = g . s  (col-packed over examples), softmax on DVE/ACT,
  PE: c = w @ enc (col-packed), out = c @ W_o (+ b_o via extra contraction row)
"""

import sys

sys.path.insert(0, "/opt/trn_rl_repo")

import numpy as np

import concourse.bass as bass
import concourse.mybir as mybir
import concourse.tile as tile
from concourse import bacc
from concourse.bass import ds, ts

# problem dims (hardcoded; kernel.py must be self-contained)
B, T = 32, 1500
E, D, A, C, K, O = 512, 512, 512, 10, 100, 512
SCALING = 2.0
NCORES = 8
BE = B // NCORES  # 4 examples per core

F32 = mybir.dt.float32
BF16 = mybir.dt.bfloat16

P = 128
TC = 12          # t chunks of 128 (11 full + partial 92); padded alloc
T_PAD = TC * P   # 1536
T_LAST = T - 11 * P  # 92
EC = E // P      # 4
AT = A // P      # 4 a tiles
T_BANKS = [(0, 512), (512, 512), (1024, 476)]  # psum-bank sized t slices
KK = 2 * K + 1   # 201 conv taps
K1N = 2          # tap chunks of 128 (128 + 73)
X_FREE = T + P   # 1628: X2b[k2, t'] = ap_pad[t' + k2], t' in [0, X_FREE)
AP_LEN = X_FREE + P  # padded att_prev row length 1756 (even)


def _build(nc: bacc.Bacc):
    # ---- DRAM I/O (per-core shard shapes) ----
    enc = nc.dram_tensor("enc_pad", [BE, T, E], F32, kind="ExternalInput").ap()
    enc_len = nc.dram_tensor("enc_len", [BE], mybir.dt.int32, kind="ExternalInput").ap()  # noqa: F841 (unused by math)
    dec_z = nc.dram_tensor("dec_z", [BE, D], F32, kind="ExternalInput").ap()
    att_prev = nc.dram_tensor("att_prev", [BE, T], F32, kind="ExternalInput").ap()
    W_enc = nc.dram_tensor("W_enc", [E, A], F32, kind="ExternalInput").ap()
    b_enc = nc.dram_tensor("b_enc", [A], F32, kind="ExternalInput").ap()
    W_dec = nc.dram_tensor("W_dec", [D, A], F32, kind="ExternalInput").ap()
    W_att = nc.dram_tensor("W_att", [C, A], F32, kind="ExternalInput").ap()
    conv_w = nc.dram_tensor("conv_w", [C, 1, KK], F32, kind="ExternalInput").ap()
    gvec_w = nc.dram_tensor("gvec_w", [A], F32, kind="ExternalInput").ap()
    W_o = nc.dram_tensor("W_o", [E, O], F32, kind="ExternalInput").ap()
    b_o = nc.dram_tensor("b_o", [O], F32, kind="ExternalInput").ap()
    out_c = nc.dram_tensor("out_c", [BE, O], F32, kind="ExternalOutput").ap()
    out_w = nc.dram_tensor("out_w", [BE, T], F32, kind="ExternalOutput").ap()
    # small DRAM scratch for partition<->free relayouts
    wtmp = nc.dram_tensor("wtmp", [BE, T_PAD], BF16, kind="Internal").ap()
    aptmp = nc.dram_tensor("aptmp", [1, BE, AP_LEN], BF16, kind="Internal").ap()
    ctmp = nc.dram_tensor("ctmp", [BE, E], F32, kind="Internal").ap()
    dtmp = nc.dram_tensor("dtmp", [BE, A], F32, kind="Internal").ap()

    with tile.TileContext(nc) as tc, \
         tc.tile_pool(name="persist", bufs=1) as pp, \
         tc.tile_pool(name="work", bufs=3) as wp, \
         tc.tile_pool(name="stgp", bufs=2) as stgp, \
         tc.tile_pool(name="ppre", bufs=4, space="PSUM") as ppre, \
         tc.tile_pool(name="pbig", bufs=1, space="PSUM") as pbig, \
         tc.tile_pool(name="pmisc", bufs=1, space="PSUM") as pmisc:

        # ================= prologue: weights =================
        # W_enc as lhsT [e, a] -> [128, ec, A] bf16
        W_enc_b = pp.tile([P, EC, A], BF16, tag="W_enc_b")
        nc.gpsimd.dma_start(W_enc_b[:], W_enc.rearrange("(ec p) a -> p ec a", p=P))
        # W_dec lhsT [d, a]
        W_dec_b = pp.tile([P, EC, A], BF16, tag="W_dec_b")
        nc.gpsimd.dma_start(W_dec_b[:], W_dec.rearrange("(dc p) a -> p dc a", p=P))
        # W_o rhs [e, o]
        W_o_b = pp.tile([P, EC, O], BF16, tag="W_o_b")
        nc.gpsimd.dma_start(W_o_b[:], W_o.rearrange("(ec p) o -> p ec o", p=P))
        # W_att padded to 32 contraction rows (rows C..32 zero)
        W_att_b = pp.tile([32, A], BF16, tag="W_att_b")
        nc.any.memzero(W_att_b[:])
        nc.gpsimd.dma_start(W_att_b[:C, :], W_att)
        # conv taps [k2, k1, c] zero-padded beyond 201; k = 128*k1 + k2
        taps = pp.tile([P, K1N, C], BF16, tag="taps")
        nc.any.memzero(taps[:])
        with nc.allow_non_contiguous_dma(reason="small conv tap relayout"):
            cw = conv_w.rearrange("c one k -> c (one k)")
            nc.gpsimd.dma_start(
                taps[:, 0, :], cw[:, 0:P].rearrange("c k2 -> k2 c")
            )
            nc.gpsimd.dma_start(
                taps[: KK - P, 1, :], cw[:, P:KK].rearrange("c k2 -> k2 c")
            )
        # g vector as [128, at] bf16 (lhsT columns)
        g_b = pp.tile([P, AT], BF16, tag="g_b")
        with nc.allow_non_contiguous_dma(reason="small g relayout"):
            nc.gpsimd.dma_start(g_b[:], gvec_w.rearrange("(at p) -> p at", p=P))
        # b_enc per-partition [128, at] f32
        b_enc_sb = pp.tile([P, AT], F32, tag="b_enc_sb")
        with nc.allow_non_contiguous_dma(reason="small b_enc relayout"):
            nc.scalar.dma_start(b_enc_sb[:], b_enc.rearrange("(at p) -> p at", p=P))
        # b_o replicated to BE rows for the final bias add (stride-0 read)
        b_o4 = pp.tile([BE, O], F32, tag="b_o4")
        with nc.allow_non_contiguous_dma(reason="broadcast read"):
            nc.scalar.dma_start(
                b_o4[:],
                bass.AP(tensor=b_o.tensor, offset=0, ap=[[0, BE], [1, O]]),
            )
        # dec_z^T [d, dc, ex] bf16, padded to 128 lhsT columns (cols BE..128
        # zero) — small-M matmuls mis-lower on HW, so keep M=128
        dec_zT = pp.tile([P, EC, P], BF16, tag="dec_zT")
        nc.any.memzero(dec_zT[:])
        with nc.allow_non_contiguous_dma(reason="small dec relayout"):
            for ex in range(BE):
                nc.gpsimd.dma_start(
                    dec_zT[:, :, ex], dec_z[ex].rearrange("(dc p) -> p dc", p=P)
                )

        # ================= prologue: att_prev -> X2b =================
        # build the padded row in SBUF, bounce to DRAM, then ONE windowed DMA
        # reads the 128 overlapping shifted copies back
        ap_row = pp.tile([1, BE, AP_LEN], BF16, tag="ap_row")
        nc.any.memzero(ap_row[:])
        nc.gpsimd.dma_start(ap_row[0, :, K : K + T], att_prev)
        nc.scalar.dma_start(aptmp, ap_row[:])
        X2b = pp.tile([P, BE, X_FREE], BF16, tag="X2b")
        windows = bass.AP(
            tensor=aptmp.tensor,
            offset=0,
            ap=[[1, P], [AP_LEN, BE], [1, X_FREE]],
        )
        with nc.allow_non_contiguous_dma(reason="overlapping shifted windows"):
            nc.scalar.dma_start(X2b[:], windows)

        # dec_b = dec_z @ W_dec as [ex, a] (M=4 padded to 128), then
        # transpose via DRAM roundtrip into per-partition bias columns
        W_dec_b2 = pp.tile([P, EC, A], BF16, tag="W_dec_b2")
        nc.vector.tensor_copy(W_dec_b2[:], W_dec_b[:])
        dec_ps = pmisc.tile([P, 512], F32, tag="misc")
        for dc in range(EC):
            nc.tensor.matmul(
                dec_ps[:, :A],
                dec_zT[:, dc, :],
                W_dec_b2[:, dc, :],
                start=(dc == 0),
                stop=(dc == EC - 1),
            )
        decb_sb = pp.tile([BE, A], F32, tag="decb_sb")
        nc.vector.tensor_copy(decb_sb[:], dec_ps[:BE, :A])
        nc.scalar.dma_start(dtmp, decb_sb[:])
        dec_col = pp.tile([P, BE, AT], F32, tag="dec_col")
        with nc.allow_non_contiguous_dma(reason="small dec_b relayout"):
            nc.scalar.dma_start(
                dec_col[:], dtmp.rearrange("b (at p) -> p b at", p=P)
            )
        # bias[p, ex, at] = b_enc + dec_b
        bias_sb = pp.tile([P, BE, AT], F32, tag="bias_sb")
        nc.vector.tensor_tensor(
            bias_sb[:],
            dec_col[:],
            b_enc_sb[:, None, :].to_broadcast((P, BE, AT)),
            mybir.AluOpType.add,
        )

        # ================= per-example pipeline =================
        # ex-outer: each example's loads/transpose/compute/softmax/context
        # overlap the next example's loads. Within (ex, at), the 3 t-banks
        # share each stationary W_enc chunk.
        conv_sb = pp.tile([32, BE, T_PAD], BF16, tag="conv_sb")
        nc.any.memzero(conv_sb[:])
        e_ps = pbig.tile([P, T_PAD], F32, tag="big")
        su3 = pp.tile([P, 4], F32, tag="su3")
        w_b = pp.tile([P, T_PAD], BF16, tag="w_b")
        nc.any.memzero(w_b[:, T:])
        c_sb = pp.tile([P, E], F32, tag="c_sb")
        nats, encTs = [], []
        for ex in range(BE):
            r = slice(32 * ex, 32 * ex + 1)
            # ---- load (HWDGE, f32) + cast (DVE) + transpose ----
            nat = pp.tile([P, EC, T_PAD], BF16, tag=f"nat{ex}")
            nc.any.memzero(nat[:, :, 11 * P :])
            for (tc0, ntc) in [(0, 4), (4, 4), (8, 3)]:
                stg = stgp.tile([P, 4, E], F32, tag="stg")
                nc.sync.dma_start(
                    stg[:, :ntc, :],
                    enc[ex, tc0 * P : (tc0 + ntc) * P, :].rearrange(
                        "(tc p) e -> p tc e", p=P
                    ),
                )
                nc.vector.tensor_copy(
                    nat[:, :, tc0 * P : (tc0 + ntc) * P].rearrange(
                        "p ec (tc i) -> p ec tc i", i=P
                    ),
                    stg[:, :ntc, :].rearrange("p tc (ec i) -> p ec tc i", i=P),
                )
            stg2 = stgp.tile([P, E], F32, tag="stg2")
            nc.sync.dma_start(stg2[:T_LAST, :], enc[ex, 11 * P : T, :])
            nc.vector.tensor_copy(
                nat[:T_LAST, :, 11 * P :].rearrange("p ec i -> p ec i"),
                stg2[:T_LAST, :].rearrange("p (ec i) -> p ec i", i=P),
            )
            encT_h = []
            for h in range(2):
                eh = pp.tile([P, 2, T_PAD], BF16, tag=f"encT{ex}_{h}")
                nc.sync.dma_start_transpose(
                    eh.rearrange("p ec (tc i) -> p (ec tc) i", i=P),
                    nat[:, 2 * h : 2 * h + 2, :].rearrange("p ec f -> p (ec f)"),
                )
                encT_h.append(eh)
            nats.append(nat)
            encTs.append(encT_h)
            # ---- conv stage 1 ----
            conv_ps = ppre.tile([P, 512], F32, tag="pre", name=f"cps{ex}")
            for (t0, bw) in T_BANKS:
                for k1 in range(K1N):
                    nc.tensor.matmul(
                        conv_ps[:C, :bw],
                        taps[:, k1, :],
                        X2b[:, ex, P * k1 + t0 : P * k1 + t0 + bw],
                        start=(k1 == 0),
                        stop=(k1 == K1N - 1),
                    )
                nc.vector.tensor_copy(
                    conv_sb[:C, ex, t0 : t0 + bw], conv_ps[:C, :bw]
                )
            # ---- main: pre-tanh, tanh, e ----
            for at in range(AT):
                pres = [
                    ppre.tile([P, 512], F32, tag="pre", name=f"pre{ex}_{at}_{x}")
                    for x in range(len(T_BANKS))
                ]
                for ec in range(EC):
                    for bi, (t0, bw) in enumerate(T_BANKS):
                        nc.tensor.matmul(
                            pres[bi][:, :bw],
                            W_enc_b[:, ec, at * P : (at + 1) * P],
                            encT_h[ec // 2][:, ec % 2, t0 : t0 + bw],
                            start=(ec == 0),
                            stop=False,
                        )
                for bi, (t0, bw) in enumerate(T_BANKS):
                    nc.tensor.matmul(
                        pres[bi][:, :bw],
                        W_att_b[:, at * P : (at + 1) * P],
                        conv_sb[:, ex, t0 : t0 + bw],
                        start=False,
                        stop=True,
                    )
                for bi, (t0, bw) in enumerate(T_BANKS):
                    s_t = wp.tile([P, 512], BF16, tag="s_t")
                    nc.scalar.activation(
                        s_t[:, :bw],
                        pres[bi][:, :bw],
                        mybir.ActivationFunctionType.Tanh,
                        bias=bias_sb[:, ex, at : at + 1],
                    )
                    nc.tensor.matmul(
                        e_ps[r, t0 : t0 + bw],
                        g_b[:, at : at + 1],
                        s_t[:, :bw],
                        start=(at == 0),
                        stop=(at == AT - 1),
                        tile_position=(0, 32 * ex),
                    )
            # ---- softmax numerator (unnormalized; |e|<=18 so f32-safe) ----
            for bi, (t0, bw) in enumerate(T_BANKS):
                nc.scalar.activation(
                    w_b[r, t0 : t0 + bw],
                    e_ps[r, t0 : t0 + bw],
                    mybir.ActivationFunctionType.Exp,
                    scale=SCALING,
                    accum_out=su3[r, bi : bi + 1],
                )
            # ---- context on unnormalized w ----
            nc.scalar.dma_start(wtmp[ex, :][None, :], w_b[r, :])
            w_colT = pp.tile([P, TC], BF16, tag=f"w_colT{ex}")
            with nc.allow_non_contiguous_dma(reason="small w relayout"):
                nc.scalar.dma_start(
                    w_colT[:], wtmp[ex].rearrange("(tc p) -> p tc", p=P)
                )
            c_ps = pmisc.tile([P, 512], F32, tag="misc", name=f"c_ps{ex}")
            for tcb in range(TC):
                nc.tensor.matmul(
                    c_ps[r, :E],
                    w_colT[:, tcb : tcb + 1],
                    nat[:, :, tcb * P : (tcb + 1) * P],
                    start=(tcb == 0),
                    stop=(tcb == TC - 1),
                    tile_position=(0, 32 * ex),
                )
            nc.vector.tensor_tensor(
                su3[r, 3:4], su3[r, 0:1], su3[r, 1:2], mybir.AluOpType.add
            )
            nc.vector.tensor_tensor(
                su3[r, 3:4], su3[r, 3:4], su3[r, 2:3], mybir.AluOpType.add
            )
            # 1/su via exp(-ln(su)) on ACT (DVE has no divide/cheap reciprocal)
            lns = wp.tile([P, 1], F32, tag="lns")
            nc.scalar.activation(
                lns[r, :], su3[r, 3:4], mybir.ActivationFunctionType.Ln
            )
            rsu = pp.tile([P, 1], F32, tag=f"rsu{ex}")
            nc.scalar.activation(
                rsu[r, :], lns[r, :], mybir.ActivationFunctionType.Exp,
                scale=-1.0,
            )
            nc.vector.tensor_tensor(
                c_sb[r, :],
                c_ps[r, :E],
                rsu[r, 0:1].to_broadcast((1, E)),
                mybir.AluOpType.mult,
            )
            nc.scalar.dma_start(ctmp[ex, :][None, :], c_sb[r, :])
            # normalized w output: numerator (bf16) * 1/su
            w_all = pp.tile([P, T], F32, tag="w_all")
            nc.vector.tensor_tensor(
                w_all[r, :],
                w_b[r, :T],
                rsu[r, 0:1].to_broadcast((1, T)),
                mybir.AluOpType.mult,
            )
            nc.scalar.dma_start(out_w[ex, :][None, :], w_all[r, :])

        cT = pp.tile([P, BE, EC], F32, tag="cT")
        with nc.allow_non_contiguous_dma(reason="small c relayout"):
            nc.scalar.dma_start(
                cT[:], ctmp.rearrange("b (ec p) -> p b ec", p=P)
            )
        cT_b = pp.tile([P, EC, P], BF16, tag="cT_b")
        nc.any.memzero(cT_b[:])
        nc.vector.tensor_copy(
            cT_b[:, :, :BE], cT.rearrange("p b e -> p e b")
        )

        # ================= out = c @ W_o + b_o =================
        W_o_b2 = pp.tile([P, EC, O], BF16, tag="W_o_b2")
        nc.vector.tensor_copy(W_o_b2[:], W_o_b[:])
        o_ps = pmisc.tile([P, 512], F32, tag="misc")
        for ec in range(EC):
            nc.tensor.matmul(
                o_ps[:, :O],
                cT_b[:, ec, :],
                W_o_b2[:, ec, :],
                start=(ec == 0),
                stop=(ec == EC - 1),
            )
        o_sb = pp.tile([BE, O], F32, tag="o_sb")
        nc.vector.tensor_tensor(
            o_sb[:], o_ps[:BE, :O], b_o4[:], mybir.AluOpType.add
        )
        nc.scalar.dma_start(out_c[:], o_sb[:])

    nc.compile()
    return nc


_CACHE = {}


def _get_nc():
    if "nc" not in _CACHE:
        nc = bacc.Bacc(
            "TRN2",
            target_bir_lowering=False,
            debug=False,
            enable_asserts=False,
            num_devices=NCORES,
        )
        _build(nc)
        _CACHE["nc"] = nc
    return _CACHE["nc"]


def make_in_maps(inputs):
    in_maps = []
    for i in range(NCORES):
        sl = slice(i * BE, (i + 1) * BE)
        m = {
            "enc_pad": np.ascontiguousarray(inputs["enc_pad"][sl], dtype=np.float32),
            "enc_len": np.ascontiguousarray(inputs["enc_len"][sl], dtype=np.int32),
            "dec_z": np.ascontiguousarray(inputs["dec_z"][sl], dtype=np.float32),
            "att_prev": np.ascontiguousarray(inputs["att_prev"][sl], dtype=np.float32),
            "W_enc": np.ascontiguousarray(inputs["W_enc"], dtype=np.float32),
            "b_enc": np.ascontiguousarray(inputs["b_enc"], dtype=np.float32),
            "W_dec": np.ascontiguousarray(inputs["W_dec"], dtype=np.float32),
            "W_att": np.ascontiguousarray(inputs["W_att"], dtype=np.float32),
            "conv_w": np.ascontiguousarray(inputs["conv_w"], dtype=np.float32),
            "gvec_w": np.ascontiguousarray(inputs["gvec_w"], dtype=np.float32),
            "W_o": np.ascontiguousarray(inputs["W_o"], dtype=np.float32),
            "b_o": np.ascontiguousarray(inputs["b_o"], dtype=np.float32),
        }
        in_maps.append(m)
    return in_maps


def kernel(**inputs):
    from concourse.bass_utils import run_bass_kernel_spmd

    nc = _get_nc()
    in_maps = make_in_maps({k: np.asarray(v) for k, v in inputs.items()})
    res = run_bass_kernel_spmd(nc, in_maps, core_ids=list(range(NCORES)))
    c = np.concatenate([r["out_c"] for r in res.results], axis=0)
    w = np.concatenate([r["out_w"] for r in res.results], axis=0)
    return (c, w)


if __name__ == "__main__":
    rng = np.random.default_rng(0)
    fake = {
        "enc_pad": rng.standard_normal((B, T, E), dtype=np.float32),
        "enc_len": np.full((B,), T, dtype=np.int32),
        "dec_z": rng.standard_normal((B, D), dtype=np.float32),
        "att_prev": rng.random((B, T), dtype=np.float32) / T,
        "W_enc": rng.standard_normal((E, A), dtype=np.float32) / np.sqrt(E),
        "b_enc": np.zeros((A,), dtype=np.float32),
        "W_dec": rng.standard_normal((D, A), dtype=np.float32) / np.sqrt(D),
        "W_att": rng.standard_normal((C, A), dtype=np.float32) / np.sqrt(C),
        "conv_w": rng.standard_normal((C, 1, KK), dtype=np.float32) * 0.05,
        "gvec_w": rng.standard_normal((A,), dtype=np.float32) / np.sqrt(A),
        "W_o": rng.standard_normal((E, O), dtype=np.float32) / np.sqrt(E),
        "b_o": np.zeros((O,), dtype=np.float32),
    }
    c, w = kernel(**fake)
    print("c", c.shape, c.dtype, "w", w.shape, w.dtype)
